# revision 1
# baseline (speedup 1.0000x reference)
"""Multi-head causal attention with RoPE for TRN2, sharded over 8 NeuronCores.

Sharding: 2-way data parallel over batch x 4-way tensor parallel over heads.
Core c handles batch c//4 and heads [4*(c%4), 4*(c%4)+4).

Per-core device kernel (all matmuls fp32r):
  phase 1: q/k/v projections from host-transposed xT; RoPE applied on
           PSUM eviction via stream_shuffle + precomputed cos/sin tables.
  phase 2: causal flash attention per head, scores^T layout [k, q]:
           no max subtraction (scores ~ N(0,1)); sum-of-exp via a ones
           column appended to V; causal handled by AP column offsets +
           0/1 mask multiply on diagonal blocks; normalization via
           reciprocal + DRAM-bounce partition broadcast.
  phase 3: o-projection emitting transposed partial output [1024, 2048].

Host: gathers 8 partials, sums per batch, transposes back.
"""
import sys

sys.path.insert(0, "/opt/trn_rl_repo")

import numpy as np
import concourse.bass as bass
import concourse.mybir as mybir
import concourse.tile as tile
from concourse import bacc
from concourse.bass_utils import run_bass_kernel_spmd

D = 1024          # d_model
H = 16            # total heads
DH = 64           # head dim
S = 2048          # sequence length
B = 2             # batch
NCORES = 8
HPC = 4           # heads per core
DHC = HPC * DH    # head dims per core = 256
ROPE_THETA = 10000.0

F32 = mybir.dt.float32
F32R = mybir.dt.float32r

NEG_SLOPE = None  # placeholder to keep linters quiet

SC = 512          # seq chunk for matmul N dim
NSC = S // SC     # 4
NJT = D // 128    # 8 contraction tiles
NST = S // 128    # 16 s-tiles

# rope row permutation within one head (64 rows):
# [a0..a15, b0..b15, a16..a31, b16..b31] with a_i = dim 2i, b_i = dim 2i+1
PERM64 = ([2 * i for i in range(16)] + [2 * i + 1 for i in range(16)]
          + [2 * i for i in range(16, 32)] + [2 * i + 1 for i in range(16, 32)])
SHUF_MASK = [(r + 16) % 32 for r in range(32)]  # a<->b swap within each 32-quadrant


def _rope_tables():
    """cos/sin tables [128, S] for the permuted 2-head row layout.

    row r: quadrant q=r//32, rr=r%32, freq f=(q%2)*16 + rr%16, a-row iff rr<16.
    ct[r, s] = cos(s * invf[f]);  st[r, s] = (-1 if a-row else +1) * sin(...).
    st is returned pre-shuffled (rows permuted by the a<->b swap) so that
    shuffle(psum * st_pre) == shuffle(psum) * st.
    """
    inv = ROPE_THETA ** (-np.arange(32, dtype=np.float64) * 2.0 / 64.0)
    pos = np.arange(S, dtype=np.float64)
    r = np.arange(128)
    q, rr = r // 32, r % 32
    f = (q % 2) * 16 + (rr % 16)
    sign = np.where(rr < 16, -1.0, 1.0)
    ang = pos[None, :] * inv[f][:, None]            # [128, S]
    ct = np.cos(ang).astype(np.float32)
    st = (sign[:, None] * np.sin(ang)).astype(np.float32)
    # pre-shuffle st rows by the quadrant-local swap
    swap = (r // 32) * 32 + (rr + 16) % 32
    st_pre = st[swap]
    return ct, st_pre


def build(repeat: int = 1):
    nc = bacc.Bacc(None, target_bir_lowering=False)

    xT = nc.dram_tensor("xT", [D, S], F32, kind="ExternalInput")
    wq = nc.dram_tensor("wq", [D, DHC], F32, kind="ExternalInput")
    wk = nc.dram_tensor("wk", [D, DHC], F32, kind="ExternalInput")
    wv = nc.dram_tensor("wv", [D, DHC], F32, kind="ExternalInput")
    wo = nc.dram_tensor("wo", [DHC, D], F32, kind="ExternalInput")
    ct = nc.dram_tensor("ct", [128, S], F32, kind="ExternalInput")
    st = nc.dram_tensor("st", [128, S], F32, kind="ExternalInput")
    msk = nc.dram_tensor("msk", [128, 128], F32, kind="ExternalInput")
    vones = nc.dram_tensor("vones", [128, HPC], F32, kind="ExternalInput")
    po = nc.dram_tensor("po", [D, S], F32, kind="ExternalOutput")
    zb = nc.dram_tensor("zb", [HPC, S], F32)  # internal bounce for Z broadcast

    with tile.TileContext(nc) as tc:
        def body(_iv=None):
            _build_body(nc, tc, xT, wq, wk, wv, wo, ct, st, msk, vones, po, zb)

        if repeat == 1:
            body()
        else:
            with tc.For_i(0, repeat, 1) as iv:
                body(iv)

    nc.compile()
    return nc


def _build_body(nc, tc, xT, wq, wk, wv, wo, ct, st, msk, vones, po, zb):
    from contextlib import ExitStack
    with ExitStack() as ctx:
        # ---- persistent pools (live through attention / o-proj) ----
        pers = ctx.enter_context(tc.tile_pool(name="pers", bufs=1))
        qk_sb = [pers.tile([128, S], F32R, tag=f"qk{i}", name=f"qk{i}") for i in range(4)]
        # qk_sb[0..1] = q tiles (head pairs 0,1), [2..3] = k tiles
        v_sb = [pers.tile([128, HPC * 65], F32R, tag=f"v{i}", name=f"v{i}") for i in range(NST)]
        ho_sb = [pers.tile([128, S], F32R, tag=f"ho{i}", name=f"ho{i}") for i in range(2)]
        wos = pers.tile([128, 2, D], F32R, tag="wos")
        msks = pers.tile([128, 128], F32R, tag="msks")
        nc.sync.dma_start(out=wos, in_=wo.rearrange("(it p) m -> p it m", p=128).bitcast(F32R))
        nc.sync.dma_start(out=msks, in_=msk[:, :].bitcast(F32R))

        psum = ctx.enter_context(tc.tile_pool(name="psum", bufs=5, space="PSUM"))
        pso_pool = ctx.enter_context(tc.tile_pool(name="pso", bufs=2, space="PSUM"))

        # ---- phase 1: projections + rope (xs/w/tables scoped here) ----
        with tc.tile_pool(name="ph1", bufs=1) as ph1, \
             tc.tile_pool(name="rtmp", bufs=3) as rtmp:
            xs = [ph1.tile([128, S], F32R, tag=f"x{j}", name=f"x{j}") for j in range(NJT)]
            for j in range(NJT):
                nc.sync.dma_start(out=xs[j], in_=xT[j * 128:(j + 1) * 128, :].bitcast(F32R))
            wqs = ph1.tile([128, NJT, DHC], F32R, tag="wqs")
            wks = ph1.tile([128, NJT, DHC], F32R, tag="wks")
            wvs = ph1.tile([128, NJT, DHC], F32R, tag="wvs")
            nc.sync.dma_start(out=wqs, in_=wq.rearrange("(j p) d -> p j d", p=128).bitcast(F32R))
            nc.sync.dma_start(out=wks, in_=wk.rearrange("(j p) d -> p j d", p=128).bitcast(F32R))
            nc.sync.dma_start(out=wvs, in_=wv.rearrange("(j p) d -> p j d", p=128).bitcast(F32R))
            cts = ph1.tile([128, S], F32, tag="cts")
            sts = ph1.tile([128, S], F32, tag="sts")
            nc.sync.dma_start(out=cts, in_=ct[:, :])
            nc.sync.dma_start(out=sts, in_=st[:, :])

            # q and k projections with fused rope eviction
            for qi, ws in ((0, wqs), (1, wks)):
                for t in range(2):          # head-pair tile
                    dst = qk_sb[qi * 2 + t]
                    for sc in range(NSC):
                        ps = psum.tile([128, SC], F32, tag="mm")
                        for j in range(NJT):
                            nc.tensor.matmul(
                                ps[:, :],
                                ws[:, j, t * 128:(t + 1) * 128],
                                xs[j][:, sc * SC:(sc + 1) * SC],
                                start=(j == 0), stop=(j == NJT - 1))
                        csl = cts[:, sc * SC:(sc + 1) * SC]
                        ssl = sts[:, sc * SC:(sc + 1) * SC]
                        t1 = rtmp.tile([128, SC], F32, tag="t1")
                        tp = rtmp.tile([128, SC], F32, tag="tp")
                        t2 = rtmp.tile([128, SC], F32, tag="t2")
                        nc.vector.tensor_mul(t1, ps[:, :], csl)
                        nc.vector.tensor_mul(tp, ps[:, :], ssl)
                        nc.vector.stream_shuffle(t2, tp, SHUF_MASK)
                        nc.vector.tensor_add(
                            dst[:, sc * SC:(sc + 1) * SC], t1, t2)

            # v projection (natural layout), with ones column per head
            for si in range(NST):
                ps = psum.tile([128, DHC], F32, tag="mm")
                for j in range(NJT):
                    nc.tensor.matmul(
                        ps[:, :],
                        xs[j][:, si * 128:(si + 1) * 128],
                        wvs[:, j, :],
                        start=(j == 0), stop=(j == NJT - 1))
                for h in range(HPC):
                    nc.vector.tensor_copy(
                        v_sb[si][:, h * 65:h * 65 + 64], ps[:, h * 64:(h + 1) * 64])
                nc.sync.dma_start(
                    out=v_sb[si].rearrange("p (h e) -> p h e", e=65)[:, :, 64:65],
                    in_=vones.rearrange("p (h e) -> p h e", e=1).bitcast(F32R))

        # ---- phase 2: attention ----
        with tc.tile_pool(name="att", bufs=3) as att, \
             tc.tile_pool(name="bcp", bufs=2) as bcp:
            for h in range(HPC):
                t, hh = h // 2, h % 2
                q_t = qk_sb[t]
                k_t = qk_sb[2 + t]
                rows = slice(hh * 64, hh * 64 + 64)
                for qc in range(NSC):
                    pso = pso_pool.tile([65, SC], F32, tag="pso")
                    nkt = 4 * qc + 4
                    for kt in range(nkt):
                        off = max(0, kt * 128 - qc * SC)
                        ps = psum.tile([128, SC], F32, tag="mm")
                        nc.tensor.matmul(
                            ps[:, off:],
                            k_t[rows, kt * 128:(kt + 1) * 128],
                            q_t[rows, qc * SC + off:(qc + 1) * SC],
                            start=True, stop=True)
                        ex = att.tile([128, SC], F32R, tag="ex")
                        nc.scalar.activation(ex[:, off:], ps[:, off:],
                                             mybir.ActivationFunctionType.Exp)
                        if kt * 128 >= qc * SC:  # diagonal block: causal 0/1 mask
                            nc.vector.tensor_mul(
                                ex[:, off:off + 128], ex[:, off:off + 128], msks)
                        nc.tensor.matmul(
                            pso[:, off:],
                            v_sb[kt][:, h * 65:h * 65 + 65],
                            ex[:, off:],
                            start=(kt == 0), stop=(kt == nkt - 1))
                    # normalize: ho = pso[0:64] / Z, Z broadcast via DRAM bounce
                    rc = bcp.tile([1, SC], F32, tag="rc")
                    nc.vector.reciprocal(rc, pso[64:65, :])
                    nc.sync.dma_start(out=zb[h:h + 1, qc * SC:(qc + 1) * SC], in_=rc)
                    bc = bcp.tile([64, SC], F32, tag="bc")
                    nc.sync.dma_start(
                        out=bc,
                        in_=zb[h:h + 1, qc * SC:(qc + 1) * SC]
                        .to_broadcast((64, SC)))
                    nc.vector.tensor_mul(
                        ho_sb[t][rows, qc * SC:(qc + 1) * SC], pso[0:64, :], bc)

        # ---- phase 3: o-projection ----
        with tc.tile_pool(name="pop", bufs=3) as pop:
            for mt in range(NJT):
                for sc in range(NSC):
                    ps = psum.tile([128, SC], F32, tag="mm")
                    for it in range(2):
                        nc.tensor.matmul(
                            ps[:, :],
                            wos[:, it, mt * 128:(mt + 1) * 128],
                            ho_sb[it][:, sc * SC:(sc + 1) * SC],
                            start=(it == 0), stop=(it == 1))
                    pe = pop.tile([128, SC], F32, tag="pe")
                    nc.vector.tensor_copy(pe, ps[:, :])
                    nc.sync.dma_start(
                        out=po[mt * 128:(mt + 1) * 128, sc * SC:(sc + 1) * SC],
                        in_=pe)


_NC_CACHE = {}


def _get_nc(repeat: int = 1):
    if repeat not in _NC_CACHE:
        _NC_CACHE[repeat] = build(repeat)
    return _NC_CACHE[repeat]


def _host_prep(q_weight, k_weight, v_weight, o_weight, in_features):
    """Build the 8 per-core input maps."""
    ct, st_pre = _rope_tables()
    mask01 = np.tril(np.ones((128, 128), dtype=np.float32))  # [k, q]: 1 if k <= q
    # mask in [k, q] layout: allow k <= q -> mask01[k, q] = (k <= q)
    mask01 = (np.arange(128)[:, None] <= np.arange(128)[None, :]).astype(np.float32)

    qw = q_weight.reshape(H, DH, D)
    kw = k_weight.reshape(H, DH, D)
    vw = v_weight.reshape(H, DH, D)

    in_maps = []
    for c in range(NCORES):
        b, g = c // 4, c % 4
        heads = list(range(4 * g, 4 * g + 4))
        wq_c = np.ascontiguousarray(
            (0.125 * qw[heads][:, PERM64, :]).reshape(DHC, D).T)
        wk_c = np.ascontiguousarray(kw[heads][:, PERM64, :].reshape(DHC, D).T)
        wv_c = np.ascontiguousarray(vw[heads].reshape(DHC, D).T)
        wo_c = np.ascontiguousarray(o_weight[:, 4 * g * DH:(4 * g + 4) * DH].T)
        xT_c = np.ascontiguousarray(in_features[b].T)
        in_maps.append({
            "xT": xT_c.astype(np.float32),
            "wq": wq_c.astype(np.float32),
            "wk": wk_c.astype(np.float32),
            "wv": wv_c.astype(np.float32),
            "wo": wo_c.astype(np.float32),
            "ct": ct, "st": st_pre, "msk": mask01,
            "vones": np.ones((128, HPC), dtype=np.float32),
        })
    return in_maps


def kernel(q_weight, k_weight, v_weight, o_weight, in_features):
    q_weight = np.asarray(q_weight, dtype=np.float32)
    k_weight = np.asarray(k_weight, dtype=np.float32)
    v_weight = np.asarray(v_weight, dtype=np.float32)
    o_weight = np.asarray(o_weight, dtype=np.float32)
    in_features = np.asarray(in_features, dtype=np.float32)

    nc = _get_nc(1)
    in_maps = _host_prep(q_weight, k_weight, v_weight, o_weight, in_features)
    res = run_bass_kernel_spmd(nc, in_maps, core_ids=list(range(NCORES)))

    out = np.empty((B, S, D), dtype=np.float32)
    for b in range(B):
        acc = res.results[4 * b]["po"].copy()
        for g in range(1, 4):
            acc += res.results[4 * b + g]["po"]
        out[b] = acc.T
    return out



# revision 2
# speedup vs baseline: 1.5766x; 1.5766x over previous
"""Multi-head causal attention with RoPE for TRN2, sharded over 8 NeuronCores. v2.

Sharding: 2-way data parallel over batch x 4-way tensor parallel over heads.
Core c handles batch c//4 and heads [4*(c%4), 4*(c%4)+4).

v2 changes vs baseline:
  - bf16 operands everywhere on the matmul path (x, w, q/k/v, ex, wo);
    fp32 PSUM accumulation throughout.
  - RoPE: ACT evicts PSUM->bf16, then shuffle + 2 muls + add in bf16 on DVE.
  - exp batched over [128, 1024] two-bank PSUM spans (fewer ACT insts).
  - Z path: ones-column in V -> Z row in pso; DMA-gather Z rows to [4, 512],
    one reciprocal_approx_fast per qc, ones-select matmul broadcast to
    [128, 512] PSUM, final normalize-mul on DVE. No DRAM bounce.
  - q-major chunk order; o-projection per qc interleaves with next qc's
    attention.
"""
import sys

sys.path.insert(0, "/opt/trn_rl_repo")

import numpy as np
import ml_dtypes
import concourse.bass as bass
import concourse.mybir as mybir
import concourse.tile as tile
from concourse import bacc
from concourse.bass_utils import run_bass_kernel_spmd

D = 1024          # d_model
H = 16            # total heads
DH = 64           # head dim
S = 2048          # sequence length
B = 2             # batch
NCORES = 8
HPC = 4           # heads per core
DHC = HPC * DH    # head dims per core = 256
ROPE_THETA = 10000.0

F32 = mybir.dt.float32
F32R = mybir.dt.float32r
BF16 = mybir.dt.bfloat16

SC = 512          # seq chunk for matmul N dim
NSC = S // SC     # 4
NJT = D // 128    # 8 contraction tiles
NST = S // 128    # 16 s-tiles

BF16NP = ml_dtypes.bfloat16

# rope row permutation within one head (64 rows):
PERM64 = ([2 * i for i in range(16)] + [2 * i + 1 for i in range(16)]
          + [2 * i for i in range(16, 32)] + [2 * i + 1 for i in range(16, 32)])
SHUF_MASK = [(r + 16) % 32 for r in range(32)]  # a<->b swap within each 32-quadrant


def _rope_tables():
    """cos/sin tables [128, S] for the permuted 2-head row layout.

    ct[r, s] = cos(s * invf[f]);  st[r, s] = (-1 if a-row else +1) * sin(...)
    Used as: rope(x) = x * ct + shuffle(x) * st   (shuffle = a<->b swap).
    """
    inv = ROPE_THETA ** (-np.arange(32, dtype=np.float64) * 2.0 / 64.0)
    pos = np.arange(S, dtype=np.float64)
    r = np.arange(128)
    q, rr = r // 32, r % 32
    f = (q % 2) * 16 + (rr % 16)
    sign = np.where(rr < 16, -1.0, 1.0)
    ang = pos[None, :] * inv[f][:, None]            # [128, S]
    ct = np.cos(ang)
    st = sign[:, None] * np.sin(ang)
    # st as used on the shuffled operand: out[r] = x[r]*ct[r] + x[swap(r)]*st[r]
    # matches r1 = x1*cos - x2*sin (a-rows, sign=-1 applied to the x2 term)
    #         r2 = x1*sin + x2*cos (b-rows: x[swap]=x1, st=+sin)
    return ct.astype(BF16NP), st.astype(BF16NP)


def build(repeat: int = 1):
    nc = bacc.Bacc(None, target_bir_lowering=False)

    xT = nc.dram_tensor("xT", [D, S], BF16, kind="ExternalInput")
    wq = nc.dram_tensor("wq", [D, DHC], BF16, kind="ExternalInput")
    wk = nc.dram_tensor("wk", [D, DHC], BF16, kind="ExternalInput")
    wv = nc.dram_tensor("wv", [D, DHC], BF16, kind="ExternalInput")
    wo = nc.dram_tensor("wo", [DHC, D], BF16, kind="ExternalInput")
    ct = nc.dram_tensor("ct", [128, S], BF16, kind="ExternalInput")
    st = nc.dram_tensor("st", [128, S], BF16, kind="ExternalInput")
    msk = nc.dram_tensor("msk", [128, 128], BF16, kind="ExternalInput")
    vones = nc.dram_tensor("vones", [128, HPC], BF16, kind="ExternalInput")
    sel = nc.dram_tensor("sel", [4, 256], F32, kind="ExternalInput")
    po = nc.dram_tensor("po", [D, S], F32, kind="ExternalOutput")

    with tile.TileContext(nc) as tc:
        def body(_iv=None):
            _build_body(nc, tc, xT, wq, wk, wv, wo, ct, st, msk, vones, sel, po)

        if repeat == 1:
            body()
        else:
            with tc.For_i(0, repeat, 1) as iv:
                body(iv)

    nc.compile()
    return nc


def _build_body(nc, tc, xT, wq, wk, wv, wo, ct, st, msk, vones, sel, po):
    from contextlib import ExitStack
    with ExitStack() as ctx:
        # ---- persistent pools ----
        pers = ctx.enter_context(tc.tile_pool(name="pers", bufs=1))
        qk_sb = [pers.tile([128, S], BF16, tag=f"qk{i}", name=f"qk{i}") for i in range(4)]
        # qk_sb[0..1] = q tiles (pairs 0,1), [2..3] = k tiles (pairs 0,1)
        v_sb = [pers.tile([128, HPC * 65], BF16, tag=f"v{i}", name=f"v{i}")
                for i in range(NST)]
        ho_sb = [pers.tile([128, S], BF16, tag=f"ho{i}", name=f"ho{i}") for i in range(2)]
        wos = pers.tile([128, 2, D], BF16, tag="wos")
        msks = pers.tile([128, 128], BF16, tag="msks")
        sels = pers.tile([4, 256], F32R, tag="sels")

        # ---- phase 1: projections + rope ----
        with tc.tile_pool(name="ph1", bufs=1) as ph1, \
             tc.tile_pool(name="rtmp", bufs=3) as rtmp, \
             tc.tile_pool(name="ps1", bufs=5, space="PSUM") as ps1:
            xh = [[ph1.tile([128, S // 2], BF16, tag=f"x{j}h{h}", name=f"x{j}h{h}")
                   for j in range(NJT)] for h in range(2)]
            wqs = ph1.tile([128, NJT, DHC], BF16, tag="wqs")
            wks = ph1.tile([128, NJT, DHC], BF16, tag="wks")
            wvs = ph1.tile([128, NJT, DHC], BF16, tag="wvs")
            cts = ph1.tile([128, S], BF16, tag="cts")
            sts = ph1.tile([128, S], BF16, tag="sts")

            # DMA issue order: first-needed first
            nc.sync.dma_start(out=wqs, in_=wq.rearrange("(j p) d -> p j d", p=128))
            for j in range(NJT):
                eng = nc.scalar if j % 2 else nc.sync
                eng.dma_start(out=xh[0][j], in_=xT[j * 128:(j + 1) * 128, 0:S // 2])
            nc.sync.dma_start(out=wks, in_=wk.rearrange("(j p) d -> p j d", p=128))
            nc.scalar.dma_start(out=cts, in_=ct[:, :])
            nc.scalar.dma_start(out=sts, in_=st[:, :])
            nc.sync.dma_start(out=wvs, in_=wv.rearrange("(j p) d -> p j d", p=128))
            nc.sync.dma_start(out=msks, in_=msk[:, :])
            for j in range(NJT):
                eng = nc.scalar if j % 2 else nc.sync
                eng.dma_start(out=xh[1][j], in_=xT[j * 128:(j + 1) * 128, S // 2:S])
            nc.sync.dma_start(out=sels, in_=sel[:, :].bitcast(F32R))
            nc.sync.dma_start(out=wos, in_=wo.rearrange("(it p) m -> p it m", p=128))

            for sc in range(NSC):
                csl = cts[:, sc * SC:(sc + 1) * SC]
                ssl = sts[:, sc * SC:(sc + 1) * SC]
                # q and k projections with rope eviction
                for qi, ws in ((0, wqs), (1, wks)):
                    for t in range(2):          # head-pair tile
                        dst = qk_sb[qi * 2 + t]
                        ps = ps1.tile([128, SC], F32, tag="mm")
                        for j in range(NJT):
                            nc.tensor.matmul(
                                ps[:, :],
                                ws[:, j, t * 128:(t + 1) * 128],
                                xh[sc // 2][j][:, (sc % 2) * SC:(sc % 2 + 1) * SC],
                                start=(j == 0), stop=(j == NJT - 1))
                        ev = rtmp.tile([128, SC], BF16, tag="ev")
                        nc.scalar.copy(ev, ps[:, :])
                        sp = rtmp.tile([128, SC], BF16, tag="sp")
                        nc.vector.stream_shuffle(sp, ev, SHUF_MASK)
                        t1 = rtmp.tile([128, SC], BF16, tag="t1")
                        t2 = rtmp.tile([128, SC], BF16, tag="t2")
                        nc.vector.tensor_mul(t1, ev, csl)
                        nc.vector.tensor_mul(t2, sp, ssl)
                        nc.vector.tensor_add(
                            dst[:, sc * SC:(sc + 1) * SC], t1, t2)
                # v projection for the 4 s-tiles of this chunk
                for si in range(sc * 4, sc * 4 + 4):
                    ps = ps1.tile([128, DHC], F32, tag="mm")
                    for j in range(NJT):
                        nc.tensor.matmul(
                            ps[:, :],
                            xh[si // 8][j][:, (si % 8) * 128:(si % 8 + 1) * 128],
                            wvs[:, j, :],
                            start=(j == 0), stop=(j == NJT - 1))
                    nc.vector.tensor_copy(
                        v_sb[si].rearrange("p (h e) -> p h e", e=65)[:, :, 0:64],
                        ps.rearrange("p (h e) -> p h e", e=64)[:, :, :])
                    nc.vector.memset(
                        v_sb[si].rearrange("p (h e) -> p h e", e=65)[:, :, 64:65],
                        1.0)

        # ---- phase 2+3: attention (q-major) with interleaved o-projection ----
        with tc.tile_pool(name="att", bufs=5) as att, \
             tc.tile_pool(name="hozp", bufs=6) as hozp, \
             tc.tile_pool(name="zp", bufs=2) as zp, \
             tc.tile_pool(name="qkb", bufs=2, space="PSUM") as qkb, \
             tc.tile_pool(name="psop", bufs=2, space="PSUM") as psop, \
             tc.tile_pool(name="bcp", bufs=2, space="PSUM") as bcp, \
             tc.tile_pool(name="pop", bufs=3) as pop:
            for qc in range(NSC):
                zq = zp.tile([4, SC], F32, tag="zq")
                hoz = []
                for h in range(HPC):
                    t, hh = h // 2, h % 2
                    q_t = qk_sb[t]
                    k_t = qk_sb[2 + t]
                    rows = slice(hh * 64, hh * 64 + 64)
                    nkt = 4 * qc + 4
                    pso = psop.tile([65, SC], F32, tag="pso")
                    for kb in range(0, nkt, 2):
                        qkps = qkb.tile([128, 2 * SC], F32, tag="qkps")
                        offs = []
                        for half, kt in enumerate((kb, kb + 1)):
                            off = max(0, kt * 128 - qc * SC)
                            offs.append(off)
                            nc.tensor.matmul(
                                qkps[:, half * SC + off:(half + 1) * SC],
                                k_t[rows, kt * 128:(kt + 1) * 128],
                                q_t[rows, qc * SC + off:(qc + 1) * SC],
                                start=True, stop=True)
                        ex = att.tile([128, 2 * SC], BF16, tag="ex")
                        if offs[0] == 0 and offs[1] == 0:
                            nc.scalar.activation(ex[:, :], qkps[:, :],
                                                 mybir.ActivationFunctionType.Exp)
                        else:
                            for half in range(2):
                                o = half * SC + offs[half]
                                nc.scalar.activation(
                                    ex[:, o:(half + 1) * SC],
                                    qkps[:, o:(half + 1) * SC],
                                    mybir.ActivationFunctionType.Exp)
                        for half, kt in enumerate((kb, kb + 1)):
                            off = offs[half]
                            if kt * 128 >= qc * SC:  # diagonal: causal 0/1 mask
                                o = half * SC + off
                                nc.vector.tensor_mul(
                                    ex[:, o:o + 128], ex[:, o:o + 128], msks)
                            nc.tensor.matmul(
                                pso[:, off:],
                                v_sb[kt][:, h * 65:h * 65 + 65],
                                ex[:, half * SC + off:(half + 1) * SC],
                                start=(kt == 0), stop=(kt == nkt - 1))
                    # evict unnormalized out + Z row; free the pso bank
                    hz = hozp.tile([65, SC], F32, tag="hz")
                    nc.vector.tensor_copy(hz, pso[:, :])
                    hoz.append(hz)
                    nc.sync.dma_start(out=zq[h:h + 1, :], in_=hz[64:65, :])
                # normalization for all 4 heads of this qc
                rz = zp.tile([4, SC], F32, tag="rz")
                nc.vector.reciprocal_approx_fast(rz, zq[:, :])
                rzr = zp.tile([4, SC], F32R, tag="rzr")
                nc.vector.tensor_copy(rzr, rz)
                for t in range(2):
                    bc = bcp.tile([128, SC], F32, tag="bc")
                    nc.tensor.matmul(
                        bc[:, :], sels[:, t * 128:(t + 1) * 128],
                        rzr[:, :], start=True, stop=True)
                    for hh in range(2):
                        rows = slice(hh * 64, hh * 64 + 64)
                        nc.vector.tensor_mul(
                            ho_sb[t][rows, qc * SC:(qc + 1) * SC],
                            hoz[2 * t + hh][0:64, :], bc[rows, :])
                # o-projection for this qc chunk
                for mt in range(NJT):
                    ps = bcp.tile([128, SC], F32, tag="bc")
                    for it in range(2):
                        nc.tensor.matmul(
                            ps[:, :],
                            wos[:, it, mt * 128:(mt + 1) * 128],
                            ho_sb[it][:, qc * SC:(qc + 1) * SC],
                            start=(it == 0), stop=(it == 1))
                    pe = pop.tile([128, SC], F32, tag="pe")
                    if qc == NSC - 1:
                        nc.any.tensor_copy(pe, ps[:, :])
                    else:
                        nc.vector.tensor_copy(pe, ps[:, :])
                    nc.sync.dma_start(
                        out=po[mt * 128:(mt + 1) * 128, qc * SC:(qc + 1) * SC],
                        in_=pe)


_NC_CACHE = {}


def _get_nc(repeat: int = 1):
    if repeat not in _NC_CACHE:
        _NC_CACHE[repeat] = build(repeat)
    return _NC_CACHE[repeat]


def _host_prep(q_weight, k_weight, v_weight, o_weight, in_features):
    """Build the 8 per-core input maps."""
    ct, st = _rope_tables()
    # mask in [k, q] layout: allow k <= q
    mask01 = (np.arange(128)[:, None] <= np.arange(128)[None, :]).astype(BF16NP)
    # sel[:, t*128+m] = 1 where row = 2t + m//64 (broadcast selector)
    sel = np.zeros((4, 256), dtype=np.float32)
    for t in range(2):
        for m in range(128):
            sel[2 * t + m // 64, t * 128 + m] = 1.0

    qw = q_weight.reshape(H, DH, D)
    kw = k_weight.reshape(H, DH, D)
    vw = v_weight.reshape(H, DH, D)

    in_maps = []
    for c in range(NCORES):
        b, g = c // 4, c % 4
        heads = list(range(4 * g, 4 * g + 4))
        wq_c = np.ascontiguousarray(
            (0.125 * qw[heads][:, PERM64, :]).reshape(DHC, D).T).astype(BF16NP)
        wk_c = np.ascontiguousarray(
            kw[heads][:, PERM64, :].reshape(DHC, D).T).astype(BF16NP)
        wv_c = np.ascontiguousarray(vw[heads].reshape(DHC, D).T).astype(BF16NP)
        wo_c = np.ascontiguousarray(
            o_weight[:, 4 * g * DH:(4 * g + 4) * DH].T).astype(BF16NP)
        xT_c = np.ascontiguousarray(in_features[b].T).astype(BF16NP)
        in_maps.append({
            "xT": xT_c, "wq": wq_c, "wk": wk_c, "wv": wv_c, "wo": wo_c,
            "ct": ct, "st": st, "msk": mask01,
            "vones": np.ones((128, HPC), dtype=BF16NP),
            "sel": sel,
        })
    return in_maps


def kernel(q_weight, k_weight, v_weight, o_weight, in_features):
    q_weight = np.asarray(q_weight, dtype=np.float32)
    k_weight = np.asarray(k_weight, dtype=np.float32)
    v_weight = np.asarray(v_weight, dtype=np.float32)
    o_weight = np.asarray(o_weight, dtype=np.float32)
    in_features = np.asarray(in_features, dtype=np.float32)

    nc = _get_nc(1)
    in_maps = _host_prep(q_weight, k_weight, v_weight, o_weight, in_features)
    res = run_bass_kernel_spmd(nc, in_maps, core_ids=list(range(NCORES)))

    out = np.empty((B, S, D), dtype=np.float32)
    for b in range(B):
        acc = res.results[4 * b]["po"].astype(np.float32)
        for g in range(1, 4):
            acc += res.results[4 * b + g]["po"]
        out[b] = acc.T
    return out
